# revision 1
# baseline (speedup 1.0000x reference)
"""GATv2 (2-layer) Trainium2 kernel, 8-core SPMD, dst-sharded edge-parallel.

Strategy:
  - Nodes padded to N_PAD=100352 = 8*12544; core c owns dst shard
    [12544c, 12544(c+1)).  Edges are routed to the core owning their dst.
  - Per core, dst space is cut into 98 windows of 128 nodes.  Edges of a
    window are packed into T_FIX tiles of 128 edges (padded with dummies).
  - Per tile: indirect-DMA gather of xl[src] rows; dst one-hot matrices are
    built on-device from host-provided dstloc arrays (is_equal vs iota);
    xr[dst] is expanded from the window's 128-row slab by matmul; e_emb via
    K=6 matmul (bias row folded); leaky-relu + att dot -> logits; exp;
    segment softmax + weighted scatter-sum via one matmul into a per-window
    PSUM accumulator carrying [slots, 128 feat + 2 den] columns.
  - Per layer: xl table = shard compute + AllGather (softmax/out need
    arbitrary src rows); xr stays shard-local.  h1 is kept transposed in
    SBUF to feed layer-2 transforms.
"""

import os
import numpy as np

import concourse.bass as bass
import concourse.bacc as bacc
import concourse.mybir as mybir
import concourse.tile as tile
from concourse.bass_utils import run_bass_kernel_spmd

N = 100000
E = 800000
D_IN = 128
HID = 8
H = 2
C = 64
HC = 128
ED = 5
NEG = 0.2
NCORE = 8
N_PAD = 100352
S = N_PAD // NCORE        # 12544 nodes per shard
W = S // 128              # 98 windows per core
F32 = mybir.dt.float32
I32 = mybir.dt.int32
EPS = 1e-10


def _install_ntff_hook():
    import contextlib
    import ctypes
    import sys
    import types

    if "antenv.axon_hooks" in sys.modules:
        return
    so_path = "/opt/axon/libaxon_pjrt.so"
    try:
        lib = ctypes.CDLL(so_path)
    except OSError:
        return
    if not hasattr(lib, "axon_start_nrt_profile"):
        return
    lib.axon_start_nrt_profile.argtypes = [ctypes.POINTER(ctypes.c_int64), ctypes.c_size_t]
    lib.axon_start_nrt_profile.restype = ctypes.c_int64
    lib.axon_stop_nrt_profile.argtypes = [ctypes.c_char_p]
    lib.axon_stop_nrt_profile.restype = ctypes.c_int64

    @contextlib.contextmanager
    def _hook(output_dir, device_ids):
        import jax

        jax.devices()
        if device_ids:
            ids = (ctypes.c_int64 * len(device_ids))(*device_ids)
            rc = lib.axon_start_nrt_profile(ids, len(device_ids))
        else:
            rc = lib.axon_start_nrt_profile(None, 0)
        if rc != 0:
            raise RuntimeError(f"axon_start_nrt_profile rc={rc}")
        try:
            yield
        finally:
            n = lib.axon_stop_nrt_profile(str(output_dir).encode())
            print(f"ntff profile: {n} file(s) -> {output_dir}", file=sys.stderr)

    mod = types.ModuleType("antenv.axon_hooks")
    _state = {"hook": _hook}
    mod.set_axon_ntff_profile_hook = lambda h: _state.__setitem__("hook", h)
    mod.get_axon_ntff_profile_hook = lambda: _state["hook"]
    sys.modules["antenv.axon_hooks"] = mod
    import antenv

    antenv.axon_hooks = mod


def _prep_edges(edge_index):
    """Per-core tile-packed edge arrays. Returns (T_FIX, per-core dict list)."""
    src = edge_index[0].astype(np.int64)
    dst = edge_index[1].astype(np.int64)
    owner = dst // S
    order = np.argsort(dst, kind="stable")
    src_s, dst_s = src[order], dst[order]
    own_s = owner[order]
    ord_s = order

    # window (0..N_PAD/128) of each sorted edge
    win = dst_s // 128
    # per (global window) counts
    cnt = np.bincount(win, minlength=N_PAD // 128)
    T_FIX = int(np.ceil(cnt.max() / 128))
    per_core = []
    for c in range(NCORE):
        sel = own_s == c
        s_c, d_c, e_c = src_s[sel], dst_s[sel], ord_s[sel]
        w_c = d_c // 128 - c * W  # local window 0..W-1
        ntile = W * T_FIX
        src_pad = np.zeros(ntile * 128, np.int32)
        dloc_pad = np.full(ntile * 128, -1.0, np.float32)
        eidx_pad = np.full(ntile * 128, -1, np.int64)  # -1 = dummy
        # edges are dst-sorted so each window's edges are contiguous
        starts = np.searchsorted(w_c, np.arange(W))
        ends = np.searchsorted(w_c, np.arange(W) + 1)
        for w in range(W):
            a, b = starts[w], ends[w]
            n = b - a
            base = w * T_FIX * 128
            src_pad[base:base + n] = s_c[a:b]
            dloc_pad[base:base + n] = (d_c[a:b] - (c * S + w * 128)).astype(np.float32)
            eidx_pad[base:base + n] = e_c[a:b]
        per_core.append({
            "src_pad": src_pad, "dloc_pad": dloc_pad, "eidx_pad": eidx_pad,
        })
    return T_FIX, per_core


def kernel(x, edge_index, edge_attr, W0, b0,
           Wl1, bl1, Wr1, br1, We1, att1, bias1,
           Wl2, bl2, Wr2, br2, We2, att2, bias2):
    x = np.asarray(x, np.float32)
    edge_index = np.asarray(edge_index, np.int32)
    edge_attr = np.asarray(edge_attr, np.float32)
    W0, b0 = np.asarray(W0, np.float32), np.asarray(b0, np.float32)
    Wl1, bl1 = np.asarray(Wl1, np.float32), np.asarray(bl1, np.float32)
    Wr1, br1 = np.asarray(Wr1, np.float32), np.asarray(br1, np.float32)
    We1, att1 = np.asarray(We1, np.float32), np.asarray(att1, np.float32)
    bias1 = np.asarray(bias1, np.float32)
    Wl2, bl2 = np.asarray(Wl2, np.float32), np.asarray(bl2, np.float32)
    Wr2, br2 = np.asarray(Wr2, np.float32), np.asarray(br2, np.float32)
    We2, att2 = np.asarray(We2, np.float32), np.asarray(att2, np.float32)
    bias2 = np.asarray(bias2, np.float32)

    T_FIX, pc = _prep_edges(edge_index)
    NT = W * T_FIX            # tiles per core
    NE = NT * 128             # padded edges per core

    # host-side constant tensors
    x_pad = np.zeros((N_PAD, D_IN), np.float32)
    x_pad[:N] = x
    xT = np.ascontiguousarray(x_pad.T)              # [128, N_PAD]
    iota_f = np.tile(np.arange(128, dtype=np.float32), (128, 1))
    iota_p = np.ascontiguousarray(iota_f.T)
    ones_r = np.ones((1, 128), np.float32)
    We1a = np.vstack([We1, (bl1 + br1)[None, :]])   # [6, 128]
    We2a = np.vstack([We2, (bl2 + br2)[None, :]])
    att1b = np.tile(att1.reshape(1, HC), (128, 1))
    att2b = np.tile(att2.reshape(1, HC), (128, 1))
    bias1b = np.tile(bias1.reshape(1, HC), (128, 1))
    bias2b = np.tile(bias2.reshape(1, HC), (128, 1))
    eps_row = np.zeros((1, 130), np.float32)
    eps_row[0, 128:] = EPS

    # per-core input arrays
    in_maps = []
    for c in range(NCORE):
        d = pc[c]
        src2d = np.ascontiguousarray(d["src_pad"].reshape(NT, 128).T).astype(np.int32)
        dcol2d = np.ascontiguousarray(d["dloc_pad"].reshape(NT, 128).T)
        drow = d["dloc_pad"].reshape(1, NE)
        ea = np.zeros((6, NE), np.float32)
        valid = d["eidx_pad"] >= 0
        ea[:5, valid] = edge_attr[d["eidx_pad"][valid]].T
        ea[5, valid] = 1.0
        in_maps.append({
            "xT": np.ascontiguousarray(xT[:, c * S:(c + 1) * S]),
            "src2d": src2d, "dcol2d": dcol2d, "drow": drow, "eattrT": ea,
            "W0": W0, "b0c": b0.reshape(HID, 1),
            "Wl1": Wl1, "Wr1": Wr1, "We1a": We1a, "att1b": att1b, "bias1b": bias1b,
            "Wl2": Wl2, "Wr2": Wr2, "We2a": We2a, "att2b": att2b, "bias2b": bias2b,
            "iota_f": iota_f, "iota_p": iota_p, "ones_r": ones_r, "eps_row": eps_row,
            "ident": np.eye(128, dtype=np.float32),
        })

    nc = bacc.Bacc("TRN2", target_bir_lowering=False, debug=False, num_devices=NCORE)

    t_xT = nc.dram_tensor("xT", [128, S], F32, kind="ExternalInput")
    t_src = nc.dram_tensor("src2d", [128, NT], I32, kind="ExternalInput")
    t_dcol = nc.dram_tensor("dcol2d", [128, NT], F32, kind="ExternalInput")
    t_drow = nc.dram_tensor("drow", [1, NE], F32, kind="ExternalInput")
    t_ea = nc.dram_tensor("eattrT", [6, NE], F32, kind="ExternalInput")
    t_W0 = nc.dram_tensor("W0", [D_IN, HID], F32, kind="ExternalInput")
    t_b0c = nc.dram_tensor("b0c", [HID, 1], F32, kind="ExternalInput")
    t_Wl1 = nc.dram_tensor("Wl1", [HID, HC], F32, kind="ExternalInput")
    t_Wr1 = nc.dram_tensor("Wr1", [HID, HC], F32, kind="ExternalInput")
    t_We1a = nc.dram_tensor("We1a", [6, HC], F32, kind="ExternalInput")
    t_att1b = nc.dram_tensor("att1b", [128, HC], F32, kind="ExternalInput")
    t_bias1b = nc.dram_tensor("bias1b", [128, HC], F32, kind="ExternalInput")
    t_Wl2 = nc.dram_tensor("Wl2", [HC, HC], F32, kind="ExternalInput")
    t_Wr2 = nc.dram_tensor("Wr2", [HC, HC], F32, kind="ExternalInput")
    t_We2a = nc.dram_tensor("We2a", [6, HC], F32, kind="ExternalInput")
    t_att2b = nc.dram_tensor("att2b", [128, HC], F32, kind="ExternalInput")
    t_bias2b = nc.dram_tensor("bias2b", [128, HC], F32, kind="ExternalInput")
    t_iota_f = nc.dram_tensor("iota_f", [128, 128], F32, kind="ExternalInput")
    t_iota_p = nc.dram_tensor("iota_p", [128, 128], F32, kind="ExternalInput")
    t_ones = nc.dram_tensor("ones_r", [1, 128], F32, kind="ExternalInput")
    t_eps = nc.dram_tensor("eps_row", [1, 130], F32, kind="ExternalInput")
    t_ident = nc.dram_tensor("ident", [128, 128], F32, kind="ExternalInput")
    t_out = nc.dram_tensor("out_shard", [S, HC], F32, kind="ExternalOutput")

    # internal DRAM
    d_xl1s = nc.dram_tensor("xl1_shard", [S, HC], F32)
    d_xr1s = nc.dram_tensor("xr1_shard", [S, HC], F32)
    d_xl1f = nc.dram_tensor("xl1_full", [N_PAD, HC], F32, addr_space="Shared")
    d_xl2s = nc.dram_tensor("xl2_shard", [S, HC], F32)
    d_xr2s = nc.dram_tensor("xr2_shard", [S, HC], F32)
    d_xl2f = nc.dram_tensor("xl2_full", [N_PAD, HC], F32, addr_space="Shared")

    AG = mybir.AluOpType.bypass
    MUL = mybir.AluOpType.mult
    ADD = mybir.AluOpType.add
    MAX = mybir.AluOpType.max
    EQ = mybir.AluOpType.is_equal
    COPY = mybir.ActivationFunctionType.Copy
    TANH = mybir.ActivationFunctionType.Tanh
    EXPF = mybir.ActivationFunctionType.Exp

    with tile.TileContext(nc) as tc:
        with tc.tile_pool(name="const", bufs=1) as cpool:
            identity = cpool.tile([128, 128], F32, tag="ident")
            nc.sync.dma_start(out=identity[:], in_=t_ident[:])
            k_iota_f = cpool.tile([128, 128], F32, tag="iota_f")
            k_iota_p = cpool.tile([128, 128], F32, tag="iota_p")
            k_ones = cpool.tile([1, 128], F32, tag="ones")
            k_eps = cpool.tile([1, 130], F32, tag="eps")
            k_W0 = cpool.tile([D_IN, HID], F32, tag="W0")
            k_b0c = cpool.tile([HID, 1], F32, tag="b0c")
            k_Wl1 = cpool.tile([HID, HC], F32, tag="Wl1")
            k_Wr1 = cpool.tile([HID, HC], F32, tag="Wr1")
            k_We1a = cpool.tile([6, HC], F32, tag="We1a")
            k_att1b = cpool.tile([128, HC], F32, tag="att1b")
            k_bias1b = cpool.tile([128, HC], F32, tag="bias1b")
            k_Wl2 = cpool.tile([HC, HC], F32, tag="Wl2")
            k_Wr2 = cpool.tile([HC, HC], F32, tag="Wr2")
            k_We2a = cpool.tile([6, HC], F32, tag="We2a")
            k_att2b = cpool.tile([128, HC], F32, tag="att2b")
            k_bias2b = cpool.tile([128, HC], F32, tag="bias2b")
            for t, srcp in [(k_iota_f, t_iota_f), (k_iota_p, t_iota_p), (k_ones, t_ones),
                            (k_eps, t_eps), (k_W0, t_W0), (k_b0c, t_b0c),
                            (k_Wl1, t_Wl1), (k_Wr1, t_Wr1), (k_We1a, t_We1a),
                            (k_att1b, t_att1b), (k_bias1b, t_bias1b),
                            (k_Wl2, t_Wl2), (k_Wr2, t_Wr2), (k_We2a, t_We2a),
                            (k_att2b, t_att2b), (k_bias2b, t_bias2b)]:
                nc.sync.dma_start(out=t[:], in_=srcp[:])

            # persistent hT [8, S] and h1T [128, S]
            hT = cpool.tile([HID, S], F32, tag="hT")
            h1T = cpool.tile([128, S], F32, tag="h1T")

            # ---------- phase 1: h = tanh(x @ W0 + b0), transposed ----------
            with (
                tc.tile_pool(name="p1s", bufs=3) as p1s,
                tc.tile_pool(name="p1p", bufs=3, space="PSUM") as p1p,
            ):
                for j in range(0, S, 448):
                    xt = p1s.tile([128, 448], F32, tag="xt")
                    nc.sync.dma_start(out=xt[:], in_=t_xT[:, j:j + 448])
                    ph = p1p.tile([HID, 448], F32, tag="ph", space="PSUM")
                    nc.tensor.matmul(out=ph[:], lhsT=k_W0[:], rhs=xt[:], start=True, stop=True)
                    nc.scalar.activation(out=hT[:, j:j + 448], in_=ph[:], func=TANH, bias=k_b0c[:, 0:1])

                # xl1/xr1 shard tables
                for t in range(W):
                    sl = slice(t * 128, (t + 1) * 128)
                    for (wmat, dram) in [(k_Wl1, d_xl1s), (k_Wr1, d_xr1s)]:
                        pxl = p1p.tile([128, HC], F32, tag="pxl", space="PSUM")
                        nc.tensor.matmul(out=pxl[:], lhsT=hT[:, sl], rhs=wmat[:], start=True, stop=True)
                        sxl = p1s.tile([128, HC], F32, tag="sxl")
                        nc.vector.tensor_copy(out=sxl[:], in_=pxl[:])
                        nc.sync.dma_start(out=dram[sl, :], in_=sxl[:])

            nc.gpsimd.collective_compute(
                "AllGather", AG, replica_groups=[list(range(NCORE))],
                ins=[d_xl1s[:]], outs=[d_xl1f[:]],
            )

            # ---------- edge phase (shared for both layers) ----------
            def edge_layer(xl_full, xr_shard, k_Wea, k_attb, k_biasb, layer):
                with (
                    tc.tile_pool(name=f"es{layer}", bufs=3) as es,
                    tc.tile_pool(name=f"ew{layer}", bufs=2) as ew,
                    tc.tile_pool(name=f"pa{layer}", bufs=2, space="PSUM") as pa,
                    tc.tile_pool(name=f"pb{layer}", bufs=2, space="PSUM") as pb,
                ):
                    for w in range(W):
                        tw0 = w * T_FIX
                        # window-level loads
                        idx_w = ew.tile([128, T_FIX], I32, tag="idx")
                        nc.sync.dma_start(out=idx_w[:], in_=t_src[:, tw0:tw0 + T_FIX])
                        dcol_w = ew.tile([128, T_FIX], F32, tag="dcol")
                        nc.sync.dma_start(out=dcol_w[:], in_=t_dcol[:, tw0:tw0 + T_FIX])
                        drow_w = ew.tile([1, T_FIX * 128], F32, tag="drow")
                        nc.sync.dma_start(out=drow_w[:], in_=t_drow[:, tw0 * 128:(tw0 + T_FIX) * 128])
                        ea_w = ew.tile([6, T_FIX * 128], F32, tag="ea")
                        nc.sync.dma_start(out=ea_w[:], in_=t_ea[:, tw0 * 128:(tw0 + T_FIX) * 128])
                        xr_w = ew.tile([128, HC], F32, tag="xr")
                        nc.sync.dma_start(out=xr_w[:], in_=xr_shard[w * 128:(w + 1) * 128, :])

                        out_ps = pb.tile([128, 130], F32, tag="outp", space="PSUM")
                        nc.tensor.matmul(out=out_ps[:], lhsT=k_ones[:], rhs=k_eps[:], start=True, stop=False)

                        for t in range(T_FIX):
                            esl = slice(t * 128, (t + 1) * 128)
                            xl_src = es.tile([128, HC], F32, tag="xls")
                            nc.gpsimd.indirect_dma_start(
                                out=xl_src[:], out_offset=None, in_=xl_full[:],
                                in_offset=bass.IndirectOffsetOnAxis(ap=idx_w[:, t:t + 1], axis=0),
                            )
                            # dstlocT broadcast [s, e]
                            ps_dT = pa.tile([128, 128], F32, tag="dT", space="PSUM")
                            nc.tensor.matmul(out=ps_dT[:], lhsT=k_ones[:], rhs=drow_w[:, esl],
                                             start=True, stop=True)
                            oh = es.tile([128, 128], F32, tag="oh")
                            nc.vector.tensor_tensor(out=oh[:], in0=dcol_w[:, t:t + 1].to_broadcast([128, 128]),
                                                    in1=k_iota_f[:], op=EQ)
                            ohT = es.tile([128, 128], F32, tag="ohT")
                            nc.vector.tensor_tensor(out=ohT[:], in0=k_iota_p[:], in1=ps_dT[:], op=EQ)

                            ps_m = pa.tile([128, HC], F32, tag="m", space="PSUM")
                            nc.tensor.matmul(out=ps_m[:], lhsT=ea_w[:, esl], rhs=k_Wea[:], start=True, stop=False)
                            nc.tensor.matmul(out=ps_m[:], lhsT=ohT[:], rhs=xr_w[:], start=False, stop=False)
                            nc.tensor.matmul(out=ps_m[:], lhsT=identity[:], rhs=xl_src[:], start=False, stop=True)

                            # leaky(x) = 0.2x + relu(0.8x): one ACT + one DVE op
                            r8 = es.tile([128, HC], F32, tag="r8")
                            nc.scalar.activation(out=r8[:], in_=ps_m[:],
                                                 func=mybir.ActivationFunctionType.Relu, scale=0.8)
                            leak = es.tile([128, HC], F32, tag="leak")
                            nc.vector.scalar_tensor_tensor(out=leak[:], in0=ps_m[:], scalar=NEG,
                                                           in1=r8[:], op0=MUL, op1=ADD)
                            lm = es.tile([128, HC], F32, tag="lm")
                            nc.vector.tensor_tensor(out=lm[:], in0=leak[:], in1=k_attb[:], op=MUL)
                            logit = es.tile([128, 2], F32, tag="logit")
                            nc.vector.tensor_reduce(out=logit[:], in_=lm[:].rearrange("p (h c) -> p h c", h=2),
                                                    axis=mybir.AxisListType.X, op=ADD)
                            ex = es.tile([128, 2], F32, tag="ex")
                            nc.scalar.activation(out=ex[:], in_=logit[:], func=EXPF)

                            w2 = es.tile([128, 130], F32, tag="w2")
                            nc.vector.tensor_tensor(out=w2[:, 0:64], in0=xl_src[:, 0:64],
                                                    in1=ex[:, 0:1].to_broadcast([128, 64]), op=MUL)
                            nc.vector.tensor_tensor(out=w2[:, 64:128], in0=xl_src[:, 64:128],
                                                    in1=ex[:, 1:2].to_broadcast([128, 64]), op=MUL)
                            nc.vector.tensor_copy(out=w2[:, 128:130], in_=ex[:])
                            nc.tensor.matmul(out=out_ps[:], lhsT=oh[:], rhs=w2[:],
                                             start=False, stop=(t == T_FIX - 1))

                        # window epilogue
                        rcp = es.tile([128, 2], F32, tag="rcp")
                        nc.vector.reciprocal(out=rcp[:], in_=out_ps[:, 128:130])
                        fin = es.tile([128, HC], F32, tag="fin")
                        nc.vector.scalar_tensor_tensor(out=fin[:, 0:64], in0=out_ps[:, 0:64],
                                                       scalar=rcp[:, 0:1], in1=k_biasb[:, 0:64],
                                                       op0=MUL, op1=ADD)
                        nc.vector.scalar_tensor_tensor(out=fin[:, 64:128], in0=out_ps[:, 64:128],
                                                       scalar=rcp[:, 1:2], in1=k_biasb[:, 64:128],
                                                       op0=MUL, op1=ADD)
                        if layer == 1:
                            ps_T = pb.tile([128, 128], F32, tag="pT", space="PSUM")
                            nc.tensor.transpose(out=ps_T[:], in_=fin[:], identity=identity[:])
                            nc.vector.tensor_copy(out=h1T[:, w * 128:(w + 1) * 128], in_=ps_T[:])
                        else:
                            fin2 = es.tile([128, HC], F32, tag="fin2")
                            nc.scalar.activation(out=fin2[:], in_=fin[:], func=TANH)
                            nc.sync.dma_start(out=t_out[w * 128:(w + 1) * 128, :], in_=fin2[:])

            edge_layer(d_xl1f, d_xr1s, k_We1a, k_att1b, k_bias1b, layer=1)

            # ---------- phase 3: layer-2 transforms ----------
            with (
                tc.tile_pool(name="p3s", bufs=3) as p3s,
                tc.tile_pool(name="p3p", bufs=3, space="PSUM") as p3p,
            ):
                for t in range(W):
                    sl = slice(t * 128, (t + 1) * 128)
                    for (wmat, dram) in [(k_Wl2, d_xl2s), (k_Wr2, d_xr2s)]:
                        pxl = p3p.tile([128, HC], F32, tag="pxl2", space="PSUM")
                        nc.tensor.matmul(out=pxl[:], lhsT=h1T[:, sl], rhs=wmat[:], start=True, stop=True)
                        sxl = p3s.tile([128, HC], F32, tag="sxl2")
                        nc.vector.tensor_copy(out=sxl[:], in_=pxl[:])
                        nc.sync.dma_start(out=dram[sl, :], in_=sxl[:])

            nc.gpsimd.collective_compute(
                "AllGather", AG, replica_groups=[list(range(NCORE))],
                ins=[d_xl2s[:]], outs=[d_xl2f[:]],
            )

            edge_layer(d_xl2f, d_xr2s, k_We2a, k_att2b, k_bias2b, layer=2)

    nc.compile()

    if os.environ.get("GAT_BUILD_ONLY"):
        return None

    trace = bool(int(os.environ.get("GAT_TRACE", "0")))
    if trace:
        _install_ntff_hook()
    res = run_bass_kernel_spmd(nc, in_maps, core_ids=list(range(NCORE)), trace=trace)
    if trace and res.exec_time_ns is not None:
        print(f"HW exec time: {res.exec_time_ns} ns")

    out = np.concatenate([res.results[c]["out_shard"] for c in range(NCORE)], axis=0)
    return np.ascontiguousarray(out[:N])


if __name__ == "__main__":
    import reference

    inputs = {k: np.asarray(v) for k, v in reference.setup_inputs().items()}
    got = kernel(**inputs)
    print("kernel output:", got.shape, got.dtype)

